# revision 2
# baseline (speedup 1.0000x reference)
"""Binary KL divergence sum on 8 Trainium2 NeuronCores.

Reference math (per element, summed over all 2**25 elements):
    kl = p*(ln p - ln q) + (1-p)*(ln(1-p) - ln(1-q))

Rewritten with t1 = ln p - ln q, t2 = ln(1-p) - ln(1-q):
    kl = t2 + p*(t1 - t2)
    sum(kl) = sum(t2) + sum(p * (t1 - t2))

Sharding: element axis split evenly across 8 cores; each core computes
per-partition partial sums; host sums the 8 * 128 * T partials.

Per-core pipeline (T tiles of [128, 2F]):
  DMA   : p chunk -> PQ[:, :F], q chunk -> PQ[:, F:]        (fp32)
  ACT   : L  = Ln(PQ)          -> fp16   (lp | lq   in one instr)
  ACT   : L1 = Ln(1 - PQ)      -> fp16   (l1p | l1q in one instr)
  DVE   : p16 = copy(PQ[:, :F])                       (fp32 -> fp16, 2x)
  DVE   : t1 = L[:, :F] - L[:, F:]                    (fp16 TT, 2x)
  DVE   : t2 = L1[:, :F] - L1[:, F:], accum -> accT2  (TTR)
  DVE   : d  = t1 - t2                                (fp16 TT, 2x)
  DVE   : p16 * d, accum -> accM                      (TTR)
  DMA   : accT2, accM -> partials [2, 128, T]
"""

import numpy as np

import concourse.bass as bass
import concourse.bacc as bacc
import concourse.mybir as mybir
from concourse import bass_utils
from concourse.tile import TileContext

N = 33554432
NCORES = 8
PER = N // NCORES  # 4194304 elements per core per tensor
P = 128
F = 2048           # half-tile free dim
T = PER // (P * F)  # 16 tiles

AF = mybir.ActivationFunctionType
OP = mybir.AluOpType
DT = mybir.dt

_NC_CACHE = {}


def _build_nc():
    nc = bacc.Bacc("TRN2", target_bir_lowering=False, debug=False,
                   num_devices=NCORES)
    inp = nc.dram_tensor("input", [PER], DT.float32, kind="ExternalInput")
    tgt = nc.dram_tensor("target", [PER], DT.float32, kind="ExternalInput")
    out = nc.dram_tensor("partials", [2, P, T], DT.float32,
                         kind="ExternalOutput")

    p_view = inp.ap().rearrange("(t p f) -> t p f", p=P, f=F)
    q_view = tgt.ap().rearrange("(t p f) -> t p f", p=P, f=F)
    out_view = out.ap()

    with TileContext(nc) as tc:
        with (
            tc.tile_pool(name="io32", bufs=3) as io32,
            tc.tile_pool(name="l16", bufs=3) as l16,
            tc.tile_pool(name="acc", bufs=1) as accp,
        ):
            accT2 = accp.tile([P, T], DT.float32, tag="accT2")
            accM = accp.tile([P, T], DT.float32, tag="accM")
            for t in range(T):
                pq = io32.tile([P, 2 * F], DT.float32, tag="pq")
                nc.sync.dma_start(pq[:, 0:F], p_view[t])
                nc.sync.dma_start(pq[:, F:2 * F], q_view[t])

                L = l16.tile([P, 2 * F], DT.float16, tag="L")
                L1 = l16.tile([P, 2 * F], DT.float16, tag="L1")
                nc.scalar.activation(L[:], pq[:], AF.Ln)
                nc.scalar.activation(L1[:], pq[:], AF.Ln, bias=1.0, scale=-1.0)

                p16 = l16.tile([P, F], DT.float16, tag="p16")
                nc.vector.tensor_copy(p16[:], pq[:, 0:F])

                t1 = l16.tile([P, F], DT.float16, tag="t1")
                nc.vector.tensor_tensor(t1[:], L[:, 0:F], L[:, F:2 * F],
                                        OP.subtract)

                t2 = l16.tile([P, F], DT.float16, tag="t2")
                nc.vector.scalar_tensor_tensor(
                    out=t2[:], in0=L1[:, 0:F], scalar=1.0,
                    in1=L1[:, F:2 * F], op0=OP.mult, op1=OP.subtract,
                    accum_out=accT2[:, t:t + 1])

                d = l16.tile([P, F], DT.float16, tag="d")
                nc.vector.tensor_tensor(d[:], t1[:], t2[:], OP.subtract)

                scr = l16.tile([P, F], DT.float16, tag="scr")
                nc.vector.scalar_tensor_tensor(
                    out=scr[:], in0=p16[:], scalar=1.0, in1=d[:],
                    op0=OP.mult, op1=OP.mult,
                    accum_out=accM[:, t:t + 1])

            nc.sync.dma_start(out_view[0], accT2[:])
            nc.sync.dma_start(out_view[1], accM[:])

    nc.compile()
    return nc


def _get_nc():
    if "nc" not in _NC_CACHE:
        _NC_CACHE["nc"] = _build_nc()
    return _NC_CACHE["nc"]


def kernel(input, target, _trace=False):
    input = np.ascontiguousarray(np.asarray(input), dtype=np.float32)
    target = np.ascontiguousarray(np.asarray(target), dtype=np.float32)
    nc = _get_nc()
    in_maps = [
        {
            "input": input[c * PER:(c + 1) * PER],
            "target": target[c * PER:(c + 1) * PER],
        }
        for c in range(NCORES)
    ]
    res = bass_utils.run_bass_kernel_spmd(
        nc, in_maps, core_ids=list(range(NCORES)), trace=_trace)
    total = np.float64(0.0)
    for c in range(NCORES):
        total += res.results[c]["partials"].astype(np.float64).sum()
    out = np.float32(total)
    if _trace:
        return out, res
    return out


# revision 3
# speedup vs baseline: 1.0464x; 1.0464x over previous
"""Binary KL divergence sum on 8 Trainium2 NeuronCores.

Reference math (per element, summed over all 2**25 elements):
    kl = p*(ln p - ln q) + (1-p)*(ln(1-p) - ln(1-q))

Rewritten with t1 = ln p - ln q, t2 = ln(1-p) - ln(1-q):
    kl = t2 + p*(t1 - t2)
    sum(kl) = sum(t2) + sum(p * (t1 - t2))

Sharding: element axis split evenly across 8 cores; each core computes
per-partition partial sums; host sums the 8 * 128 * T partials.

Per-core pipeline (T tiles of [128, 2F]):
  DMA   : p chunk -> PQ[:, :F], q chunk -> PQ[:, F:]        (fp32)
  ACT   : L  = Ln(PQ)          -> fp16   (lp | lq   in one instr)
  ACT   : L1 = Ln(1 - PQ)      -> fp16   (l1p | l1q in one instr)
  DVE   : p16 = copy(PQ[:, :F])                       (fp32 -> fp16, 2x)
  DVE   : t1 = L[:, :F] - L[:, F:]                    (fp16 TT, 2x)
  DVE   : t2 = L1[:, :F] - L1[:, F:], accum -> accT2  (TTR)
  DVE   : d  = t1 - t2                                (fp16 TT, 2x)
  DVE   : p16 * d, accum -> accM                      (TTR)
  DMA   : accT2, accM -> partials [2, 128, T]
"""

import numpy as np

import concourse.bass as bass
import concourse.bacc as bacc
import concourse.mybir as mybir
from concourse import bass_utils
from concourse.tile import TileContext

N = 33554432
NCORES = 8
PER = N // NCORES  # 4194304 elements per core per tensor
P = 128
F = 2048           # half-tile free dim
T = PER // (P * F)  # 16 tiles

AF = mybir.ActivationFunctionType
OP = mybir.AluOpType
DT = mybir.dt

_NC_CACHE = {}


NRED = 512  # one PSUM bank of fp32: matmul free-dim chunk


def _build_nc():
    nc = bacc.Bacc("TRN2", target_bir_lowering=False, debug=False,
                   num_devices=NCORES)
    inp = nc.dram_tensor("input", [PER], DT.float32, kind="ExternalInput")
    tgt = nc.dram_tensor("target", [PER], DT.float32, kind="ExternalInput")
    out = nc.dram_tensor("partials", [NRED], DT.float32,
                         kind="ExternalOutput")

    p_view = inp.ap().rearrange("(t p f) -> t p f", p=P, f=F)
    q_view = tgt.ap().rearrange("(t p f) -> t p f", p=P, f=F)
    out_view = out.ap().rearrange("(o n) -> o n", o=1)

    n_chunks = F // NRED
    n_mm = T * 2 * n_chunks  # total matmuls in the accumulation group

    with TileContext(nc) as tc:
        with (
            tc.tile_pool(name="io32", bufs=3) as io32,
            tc.tile_pool(name="l16", bufs=3) as l16,
            tc.tile_pool(name="cst", bufs=1) as cst,
            tc.tile_pool(name="ps", bufs=1, space="PSUM") as psp,
        ):
            ones = cst.tile([P, 1], DT.float16, tag="ones")
            nc.vector.memset(ones[:], 1.0)
            acc = psp.tile([1, NRED], DT.float32, tag="acc")
            osb = cst.tile([1, NRED], DT.float32, tag="osb")

            mm = 0

            def mm_accum(src):
                nonlocal mm
                for c in range(n_chunks):
                    nc.tensor.matmul(
                        acc[:, :], ones[:], src[:, c * NRED:(c + 1) * NRED],
                        start=(mm == 0), stop=(mm == n_mm - 1))
                    mm += 1

            for t in range(T):
                pq = io32.tile([P, 2 * F], DT.float32, tag="pq")
                nc.sync.dma_start(pq[:, 0:F], p_view[t])
                nc.sync.dma_start(pq[:, F:2 * F], q_view[t])

                L = l16.tile([P, 2 * F], DT.float16, tag="L")
                L1 = l16.tile([P, 2 * F], DT.float16, tag="L1")
                nc.scalar.activation(L[:], pq[:], AF.Ln)
                nc.scalar.activation(L1[:], pq[:], AF.Ln, bias=1.0, scale=-1.0)

                p16 = l16.tile([P, F], DT.float16, tag="p16")
                nc.vector.tensor_copy(p16[:], pq[:, 0:F])

                t1 = l16.tile([P, F], DT.float16, tag="t1")
                nc.vector.tensor_tensor(t1[:], L[:, 0:F], L[:, F:2 * F],
                                        OP.subtract)

                t2 = l16.tile([P, F], DT.float16, tag="t2")
                nc.vector.tensor_tensor(t2[:], L1[:, 0:F], L1[:, F:2 * F],
                                        OP.subtract)

                d = l16.tile([P, F], DT.float16, tag="d")
                nc.vector.tensor_tensor(d[:], t1[:], t2[:], OP.subtract)

                m = l16.tile([P, F], DT.float16, tag="m")
                nc.vector.tensor_tensor(m[:], p16[:], d[:], OP.mult)

                mm_accum(t2)  # sum(t2)
                mm_accum(m)   # sum(p*(t1-t2))

            nc.vector.tensor_copy(osb[:], acc[:])
            nc.sync.dma_start(out_view[:], osb[:])

    nc.compile()
    return nc


def _get_nc():
    if "nc" not in _NC_CACHE:
        _NC_CACHE["nc"] = _build_nc()
    return _NC_CACHE["nc"]


def kernel(input, target, _trace=False):
    input = np.ascontiguousarray(np.asarray(input), dtype=np.float32)
    target = np.ascontiguousarray(np.asarray(target), dtype=np.float32)
    nc = _get_nc()
    in_maps = [
        {
            "input": input[c * PER:(c + 1) * PER],
            "target": target[c * PER:(c + 1) * PER],
        }
        for c in range(NCORES)
    ]
    res = bass_utils.run_bass_kernel_spmd(
        nc, in_maps, core_ids=list(range(NCORES)), trace=_trace)
    total = np.float64(0.0)
    for c in range(NCORES):
        total += res.results[c]["partials"].astype(np.float64).sum()
    out = np.float32(total)
    if _trace:
        return out, res
    return out


# revision 5
# speedup vs baseline: 1.0792x; 1.0313x over previous
"""Binary KL divergence sum on 8 Trainium2 NeuronCores.

Reference math (per element, summed over all 2**25 elements):
    kl = p*(ln p - ln q) + (1-p)*(ln(1-p) - ln(1-q))

Rewritten with t1 = ln p - ln q, t2 = ln(1-p) - ln(1-q):
    kl = t2 + p*(t1 - t2)
    sum(kl) = sum(t2) + sum(p * (t1 - t2))

Sharding: element axis split evenly across 8 cores; each core computes
per-partition partial sums; host sums the 8 * 128 * T partials.

Per-core pipeline (T tiles of [128, 2F]):
  DMA   : p chunk -> PQ[:, :F], q chunk -> PQ[:, F:]        (fp32)
  ACT   : L  = Ln(PQ)          -> fp16   (lp | lq   in one instr)
  ACT   : L1 = Ln(1 - PQ)      -> fp16   (l1p | l1q in one instr)
  DVE   : p16 = copy(PQ[:, :F])                       (fp32 -> fp16, 2x)
  DVE   : t1 = L[:, :F] - L[:, F:]                    (fp16 TT, 2x)
  DVE   : t2 = L1[:, :F] - L1[:, F:], accum -> accT2  (TTR)
  DVE   : d  = t1 - t2                                (fp16 TT, 2x)
  DVE   : p16 * d, accum -> accM                      (TTR)
  DMA   : accT2, accM -> partials [2, 128, T]
"""

import numpy as np

import concourse.bass as bass
import concourse.bacc as bacc
import concourse.mybir as mybir
from concourse import bass_utils
from concourse.tile import TileContext

N = 33554432
NCORES = 8
PER = N // NCORES  # 4194304 elements per core per tensor
P = 128
F = 2048           # half-tile free dim
T = PER // (P * F)  # 16 tiles

AF = mybir.ActivationFunctionType
OP = mybir.AluOpType
DT = mybir.dt

_NC_CACHE = {}


NRED = 512  # one PSUM bank of fp32: matmul free-dim chunk
# chunk schedule (free-dim widths per partition): small chunks at the start
# (ACT begins after ~0.5 MB of DMA instead of 4 MB) and at the end (short
# DVE/PE tail after the last LN).
CHUNKS = [512] * 4 + [1024] * 2 + [2048] * 13 + [512] * 4
assert sum(CHUNKS) == PER // P


def _build_nc():
    nc = bacc.Bacc("TRN2", target_bir_lowering=False, debug=False,
                   num_devices=NCORES)
    inp = nc.dram_tensor("input", [PER], DT.float32, kind="ExternalInput")
    tgt = nc.dram_tensor("target", [PER], DT.float32, kind="ExternalInput")
    out = nc.dram_tensor("partials", [NRED], DT.float32,
                         kind="ExternalOutput")

    p_flat = inp.ap()
    q_flat = tgt.ap()
    out_view = out.ap().rearrange("(o n) -> o n", o=1)

    n_mm = 2 * sum(w // NRED for w in CHUNKS)

    with TileContext(nc) as tc:
        with (
            tc.tile_pool(name="io32", bufs=3) as io32,
            tc.tile_pool(name="l16", bufs=3) as l16,
            tc.tile_pool(name="cst", bufs=1) as cst,
            tc.tile_pool(name="ps", bufs=1, space="PSUM") as psp,
        ):
            ones = cst.tile([P, 1], DT.float16, tag="ones")
            nc.vector.memset(ones[:], 1.0)
            acc = psp.tile([1, NRED], DT.float32, tag="acc")
            osb = cst.tile([1, NRED], DT.float32, tag="osb")

            # Dummy 1-element Ln at t=0 so the ACT table load happens while
            # the first DMA is still in flight. Output goes to osb (live
            # tensor, overwritten later) so DCE keeps it.
            warm = cst.tile([1, 1], DT.float32, tag="warm")
            nc.vector.memset(warm[:], 0.5)
            nc.scalar.activation(osb[0:1, 0:1], warm[:], AF.Ln)

            mm = 0

            def mm_accum(src, w):
                nonlocal mm
                for c in range(w // NRED):
                    nc.tensor.matmul(
                        acc[:, :], ones[:], src[:, c * NRED:(c + 1) * NRED],
                        start=(mm == 0), stop=(mm == n_mm - 1))
                    mm += 1

            base = 0
            for F in CHUNKS:
                pq = io32.tile([P, 2 * F], DT.float32, tag="pq")
                nc.sync.dma_start(
                    pq[:, 0:F],
                    p_flat[base:base + P * F].rearrange("(p f) -> p f", p=P))
                nc.sync.dma_start(
                    pq[:, F:2 * F],
                    q_flat[base:base + P * F].rearrange("(p f) -> p f", p=P))
                base += P * F

                L = l16.tile([P, 2 * F], DT.float16, tag="L")
                L1 = l16.tile([P, 2 * F], DT.float16, tag="L1")
                nc.scalar.activation(L[:], pq[:], AF.Ln)
                nc.scalar.activation(L1[:], pq[:], AF.Ln, bias=1.0, scale=-1.0)

                p16 = l16.tile([P, F], DT.float16, tag="p16")
                nc.vector.tensor_copy(p16[:], pq[:, 0:F])

                t1 = l16.tile([P, F], DT.float16, tag="t1")
                nc.vector.tensor_tensor(t1[:], L[:, 0:F], L[:, F:2 * F],
                                        OP.subtract)

                t2 = l16.tile([P, F], DT.float16, tag="t2")
                nc.vector.tensor_tensor(t2[:], L1[:, 0:F], L1[:, F:2 * F],
                                        OP.subtract)

                d = l16.tile([P, F], DT.float16, tag="d")
                nc.vector.tensor_tensor(d[:], t1[:], t2[:], OP.subtract)

                m = l16.tile([P, F], DT.float16, tag="m")
                nc.vector.tensor_tensor(m[:], p16[:], d[:], OP.mult)

                mm_accum(t2, F)  # sum(t2)
                mm_accum(m, F)   # sum(p*(t1-t2))

            nc.vector.tensor_copy(osb[:], acc[:])
            nc.sync.dma_start(out_view[:], osb[:])

    nc.compile()
    return nc


def _get_nc():
    if "nc" not in _NC_CACHE:
        _NC_CACHE["nc"] = _build_nc()
    return _NC_CACHE["nc"]


def kernel(input, target, _trace=False):
    input = np.ascontiguousarray(np.asarray(input), dtype=np.float32)
    target = np.ascontiguousarray(np.asarray(target), dtype=np.float32)
    nc = _get_nc()
    in_maps = [
        {
            "input": input[c * PER:(c + 1) * PER],
            "target": target[c * PER:(c + 1) * PER],
        }
        for c in range(NCORES)
    ]
    res = bass_utils.run_bass_kernel_spmd(
        nc, in_maps, core_ids=list(range(NCORES)), trace=_trace)
    total = np.float64(0.0)
    for c in range(NCORES):
        total += res.results[c]["partials"].astype(np.float64).sum()
    out = np.float32(total)
    if _trace:
        return out, res
    return out
